# revision 20
# baseline (speedup 1.0000x reference)
"""Trainium2 Bass kernel for nn_ContrastiveDist (supervised contrastive loss).

Math
----
The reference builds (n,n) distance/weight matrices, but the loss collapses
exactly to per-class statistics.  With classes c = 0..15, per-class count
cnt[c], feature sums C[c,:], squared-norm sums SqSum[c], global sums
Ftot / SSall:

    alpha[c] = 1/(cnt[c]-1+eps)
    beta[c]  = 1/(n-cnt[c]+eps)
    loss_i   = sq_i*P[c_i] + (Q[c_i]+M) + f_i . R[c_i]
      P[c]   = alpha*cnt - beta*(n-cnt)
      Q[c]   = alpha*SqSum[c] - beta*(SSall-SqSum[c])
      R[c,:] = 2*beta*(Ftot-C[c]) - 2*alpha*C[c]
    result   = sum(relu(loss_i)*valid_i) / max(sum(valid_i), 1)

Everything that depends only on the labels (cnt, alpha, beta, P, the
validity mask and the final denominator) is precomputed on the host, like
the one-hot encodings; the R/Q coefficients and all per-row work depend on
the features and stay on device.  valid_i is folded into the coefficients:
R/P/Q rows of invalid classes are zeroed so relu(loss) = 0 there.

Precision: feature path is single-chain bf16 with fp32 PSUM accumulation;
rel err vs the fp32 reference ~1.3e-3 (harness gate 2e-2).

Perf structure (per core, inputs replicated -- collectives measured at
~45us/call under this dispatch path, so none are used):
  - bf16 F in row-tile layout (fmain) AND pre-transposed layout (ftin),
    all on the sync HWDGE ring so no compute-engine sequencer ever stalls
    on a DMA issue; side tensors ride the gpsimd SWDGE ring
  - sq: ~1/3 of tiles as per-tile Square+accum_out on Scalar, the rest as
    chunked square+reduce on DVE -- the two engines run in parallel,
    pipelined behind the load
  - stats: two accumulating matmuls per tile (C from fmain, SqS from the
    sq column), chunk-pipelined behind the load
  - loss: G_t = F_t^T-layout matmuls with constant stationary weights R^T
    (27ns/tile streams), then chunked (G*onehot) pick + innermost reduce
    on DVE; sq*P + QM terms via two one-hot pick passes with the
    middle-dim stride-0 broadcast
"""

import numpy as np
import ml_dtypes

import concourse.bacc as bacc
import concourse.tile as tile
import concourse.mybir as mybir
from concourse.bass_utils import run_bass_kernel_spmd

N, D, K, NCORES = 8192, 128, 16, 8
T = N // 128               # 64 row-tiles of 128
EPS, MARGIN = 1e-6, 10.0
F32 = mybir.dt.float32
BF16 = mybir.dt.bfloat16
Alu = mybir.AluOpType
Act = mybir.ActivationFunctionType
AxX = mybir.AxisListType.X

# cst (128, CW) f32:
#   col 0      1.0 everywhere (ones(128,1) lhsT for the final reduce)
#   cols 1:17  1.0 in rows 0:16 (ones(16,16) lhsT for global sums)
#   col 18     row 0: 1/max(sum(valid),1)  (final denominator reciprocal)
#   col 19     rows 0:16: -2*alpha*vmask
#   col 20     rows 0:16:  2*beta*vmask
#   col 21     rows 0:16:  alpha*vmask
#   col 22     rows 0:16:  beta*vmask
#   col 23     rows 0:16:  MARGIN*vmask
CW = 24
# cstb (16, 160) bf16: cols 0:128 ones; 128:144 I16; 144:160 diag(P*vmask)
CBW = 160

_CACHE: dict = {}


def _build():
    if "nc" in _CACHE:
        return _CACHE["nc"]

    nc = bacc.Bacc("TRN2", target_bir_lowering=False, debug=False, num_devices=NCORES)
    fmin_ = nc.dram_tensor("fmain", [128, T * (D + 2)], BF16,
                           kind="ExternalInput").ap()
    ftin = nc.dram_tensor("ftin", [128, T * D], BF16, kind="ExternalInput").ap()
    eohin = nc.dram_tensor("eohin", [128, T * 16], BF16, kind="ExternalInput").ap()
    cst = nc.dram_tensor("cst", [128, CW], F32, kind="ExternalInput").ap()
    cstbin = nc.dram_tensor("cstb", [16, CBW], BF16, kind="ExternalInput").ap()
    res = nc.dram_tensor("res", [1, 1], F32, kind="ExternalOutput").ap()

    with tile.TileContext(nc) as tc:
        with (
            tc.tile_pool(name="sb", bufs=1) as sb,
            tc.tile_pool(name="ps", bufs=1, space="PSUM") as ps,
        ):
            # ---------------- loads ----------------
            # small side tensors on the scalar HWDGE ring (3 small issues
            # don't saturate it); all big loads on the sync ring; gpsimd
            # carries no instructions at all
            csts = sb.tile([128, CW], F32)
            nc.scalar.dma_start(csts[:], cst)
            cstb = sb.tile([16, CBW], BF16)
            nc.scalar.dma_start(cstb[:], cstbin)
            eoh = sb.tile([128, T * 16], BF16)
            nc.scalar.dma_start(eoh[:], eohin)

            WM = D + 2          # fmain stride: [F(128), sq, pad]
            fmain = sb.tile([128, T * WM], BF16)
            fm3 = fmain.rearrange("p (t d) -> p t d", d=WM)
            # (start_tile, n_tiles, square_engine)
            chunks = [(0, 6, "dve"), (6, 6, "act"), (12, 12, "dve"),
                      (24, 7, "act"), (31, 11, "dve"), (42, 7, "act"),
                      (49, 15, "dve")]
            for g, (t0, ntl, _) in enumerate(chunks):
                nc.sync.dma_start(fmain[:, t0 * WM:(t0 + ntl) * WM],
                                  fmin_[:, t0 * WM:(t0 + ntl) * WM])
            faT = sb.tile([128, T * D], BF16)
            faT3 = faT.rearrange("p (t r) -> p t r", r=128)
            for x in range(4):
                t0 = x * 16
                nc.sync.dma_start(faT[:, t0 * D:(t0 + 16) * D],
                                  ftin[:, t0 * D:(t0 + 16) * D])

            eoh3 = eoh.rearrange("p (t c) -> p t c", c=16)

            # ---------- sq + per-class stats, chunk-pipelined with load ----
            sqd = sb.tile([128, T], F32)
            statsP = ps.tile([16, D + 1], F32)
            for g, (t0, ntl, sqeng) in enumerate(chunks):
                if sqeng == "act":
                    for j in range(ntl):
                        t = t0 + j
                        ascr = sb.tile([128, D], BF16, tag="ascr", bufs=2,
                                       name=f"as{t}")
                        nc.scalar.activation(ascr[:], fm3[:, t, 0:D],
                                             Act.Square,
                                             accum_out=sqd[:, t:t + 1])
                else:
                    scr = sb.tile([128, ntl * D], BF16, tag="sqscr", bufs=2,
                                  name=f"scr{g}")
                    scr3 = scr.rearrange("p (t d) -> p t d", d=D)
                    nc.vector.tensor_tensor(scr3[:, :, :],
                                            fm3[:, t0:t0 + ntl, 0:D],
                                            fm3[:, t0:t0 + ntl, 0:D],
                                            op=Alu.mult)
                    nc.vector.tensor_reduce(sqd[:, t0:t0 + ntl], scr3,
                                            axis=AxX, op=Alu.add)
                nc.vector.tensor_copy(fm3[:, t0:t0 + ntl, D],
                                      sqd[:, t0:t0 + ntl])
                for j in range(ntl):
                    t = t0 + j
                    nc.tensor.matmul(statsP[:], eoh3[:, t, :],
                                     fm3[:, t, 0:D + 1],
                                     start=(t == 0), stop=(t == T - 1))
            stats = sb.tile([16, D + 1], F32)
            nc.vector.tensor_copy(stats[:], statsP[:])

            # ---------------- per-class coefficients ----------------
            C = stats[:, 0:D]
            SqS = stats[:, D:D + 1]
            gbP = ps.tile([16, D + 1], F32)
            nc.tensor.matmul(gbP[:], csts[0:16, 1:17], stats[:],
                             start=True, stop=True)
            gb = sb.tile([16, D + 1], F32)
            nc.vector.tensor_copy(gb[:], gbP[:])
            Ftot = gb[:, 0:D]
            SSall = gb[:, D:D + 1]

            na2 = csts[0:16, 19:20]
            b2 = csts[0:16, 20:21]
            av = csts[0:16, 21:22]
            bv = csts[0:16, 22:23]
            mv = csts[0:16, 23:24]

            raug = sb.tile([16, D], F32)
            tmpd = sb.tile([16, D], F32)
            nc.vector.tensor_tensor(tmpd[:], Ftot, C, op=Alu.subtract)
            nc.vector.tensor_scalar(tmpd[:], tmpd[:], b2, None, op0=Alu.mult)
            nc.vector.scalar_tensor_tensor(raug[:], C, na2, tmpd[:],
                                           op0=Alu.mult, op1=Alu.add)
            ssd = sb.tile([16, 1], F32)
            nc.vector.tensor_tensor(ssd[:], SSall, SqS, op=Alu.subtract)
            nc.vector.tensor_scalar(ssd[:], ssd[:], bv, None, op0=Alu.mult)
            qa = sb.tile([16, 1], F32)
            nc.vector.scalar_tensor_tensor(qa[:], SqS, av, ssd[:],
                                           op0=Alu.mult, op1=Alu.subtract)
            nc.vector.tensor_scalar(qa[:], qa[:], mv, None, op0=Alu.add)

            # R^T (128, 16) via PE transpose on bf16 values (exact);
            # [P|QM] broadcast to all 128 partitions via bf16 ones-matmul
            rbf = sb.tile([16, D], BF16)
            nc.vector.tensor_copy(rbf[:], raug[:])
            rtP = ps.tile([128, 16], BF16)
            nc.tensor.transpose(rtP[:], rbf[:], cstb[:, 128:144])
            rtb = sb.tile([128, 16], BF16)
            nc.vector.tensor_copy(rtb[:], rtP[:])

            pqd = sb.tile([16, 32], BF16)
            nc.vector.tensor_copy(pqd[:, 0:16], cstb[:, 144:160])
            nc.vector.tensor_scalar(pqd[:, 16:32], cstb[:, 128:144],
                                    qa[:], None, op0=Alu.mult)
            pqbP = ps.tile([128, 32], F32)
            nc.tensor.matmul(pqbP[:], cstb[:, 0:128], pqd[:],
                             start=True, stop=True)
            pqb = sb.tile([128, 32], BF16)
            nc.vector.tensor_copy(pqb[:], pqbP[:])

            # ---------------- per-row losses ----------------
            # Prow[p,t] = P[label], QMrow[p,t] = QM[label]: one-hot picks
            # with the middle-dim stride-0 broadcast
            pb3 = pqb[:, 0:16].unsqueeze(1).broadcast_to((128, T, 16))
            qb3 = pqb[:, 16:32].unsqueeze(1).broadcast_to((128, T, 16))
            pscr = sb.tile([128, T * 16], BF16)
            pscr3 = pscr.rearrange("p (t c) -> p t c", c=16)
            nc.vector.tensor_tensor(pscr3[:, :, :], eoh3, pb3, op=Alu.mult)
            prow = sb.tile([128, T], F32)
            nc.vector.tensor_reduce(prow[:], pscr3, axis=AxX, op=Alu.add)
            qscr = sb.tile([128, T * 16], BF16)
            qscr3 = qscr.rearrange("p (t c) -> p t c", c=16)
            nc.vector.tensor_tensor(qscr3[:, :, :], eoh3, qb3, op=Alu.mult)
            qmrow = sb.tile([128, T], F32)
            nc.vector.tensor_reduce(qmrow[:], qscr3, axis=AxX, op=Alu.add)

            # G3 chunks: G[p,t,c] = f_(t,p) . R[c]
            GCH = 32
            gpick = sb.tile([128, T], F32)
            for g in range(T // GCH):
                t0 = g * GCH
                gP = ps.tile([128, GCH * 16], F32, tag="gpsum", bufs=2,
                             name=f"gP{g}")
                gP3 = gP.rearrange("p (t c) -> p t c", c=16)
                for j in range(GCH):
                    nc.tensor.matmul(gP[:, j * 16:(j + 1) * 16],
                                     faT3[:, t0 + j, :], rtb[:],
                                     start=True, stop=True)
                pick = sb.tile([128, GCH * 16], F32, tag="pick", bufs=2,
                               name=f"pick{g}")
                pick3 = pick.rearrange("p (t c) -> p t c", c=16)
                nc.vector.tensor_tensor(pick3[:, :, :], gP3,
                                        eoh3[:, t0:t0 + GCH, :], op=Alu.mult)
                nc.vector.tensor_reduce(gpick[:, t0:t0 + GCH], pick3,
                                        axis=AxX, op=Alu.add)

            # lossrows = gpick + sq*Prow + QMrow
            lossrows = sb.tile([128, T], F32)
            nc.vector.tensor_tensor(lossrows[:], sqd[:], prow[:], op=Alu.mult)
            nc.vector.tensor_tensor(lossrows[:], lossrows[:], qmrow[:],
                                    op=Alu.add)
            nc.vector.tensor_tensor(lossrows[:], lossrows[:], gpick[:],
                                    op=Alu.add)

            # ---------------- final reduction ----------------
            acc1 = sb.tile([128, 1], F32)
            relscr = sb.tile([128, T], F32)
            nc.vector.tensor_scalar(relscr[:], lossrows[:], 0.0, None,
                                    op0=Alu.max, op1=Alu.add,
                                    accum_out=acc1[:])
            finP = ps.tile([1, 1], F32)
            nc.tensor.matmul(finP[:], csts[:, 0:1], acc1[:],
                             start=True, stop=True)
            fin = sb.tile([1, 1], F32)
            nc.vector.tensor_copy(fin[:], finP[:])
            resS = sb.tile([1, 1], F32)
            nc.vector.tensor_tensor(resS[:], fin[:], csts[0:1, 18:19],
                                    op=Alu.mult)
            nc.sync.dma_start(res, resS[:])

    nc.compile()
    _CACHE["nc"] = nc
    return nc


def _make_in_maps(features, labels):
    feats = np.ascontiguousarray(np.asarray(features, dtype=np.float32))
    lab = np.ascontiguousarray(np.asarray(labels)).astype(np.int64)

    # label-only quantities (same class of preprocessing as the one-hots)
    cnt = np.bincount(lab, minlength=16)[0:16].astype(np.float64)
    alpha = 1.0 / (cnt - 1 + EPS)
    beta = 1.0 / (N - cnt + EPS)
    P = alpha * cnt - beta * (N - cnt)
    vmask = (cnt >= 2).astype(np.float64)
    valid = float((cnt * vmask).sum())
    invden = 1.0 / max(valid, 1.0)

    cst = np.zeros((128, CW), np.float32)
    cst[:, 0] = 1.0
    cst[0:16, 1:17] = 1.0
    cst[0, 18] = invden
    cst[0:16, 19] = (-2.0 * alpha * vmask).astype(np.float32)
    cst[0:16, 20] = (2.0 * beta * vmask).astype(np.float32)
    cst[0:16, 21] = (alpha * vmask).astype(np.float32)
    cst[0:16, 22] = (beta * vmask).astype(np.float32)
    cst[0:16, 23] = (MARGIN * vmask).astype(np.float32)

    cstb = np.zeros((16, CBW), np.float32)
    cstb[:, 0:128] = 1.0
    cstb[:, 128:144] = np.eye(16, dtype=np.float32)
    cstb[:, 144:160] = np.diag((P * vmask).astype(np.float32))
    cstb = cstb.astype(ml_dtypes.bfloat16)

    WM = D + 2
    fmain = np.zeros((128, T, WM), np.float32)
    fmain[:, :, 0:D] = feats.reshape(T, 128, D).transpose(1, 0, 2)
    fmain = fmain.reshape(128, T * WM).astype(ml_dtypes.bfloat16)
    ftin = np.ascontiguousarray(
        feats.reshape(T, 128, D).transpose(2, 0, 1).reshape(128, T * 128)
    ).astype(ml_dtypes.bfloat16)

    labT = lab.reshape(T, 128).T                       # (128, T)
    eoh = (labT[:, :, None] == np.arange(16)[None, None, :])
    eohin = np.ascontiguousarray(
        eoh.reshape(128, T * 16)).astype(ml_dtypes.bfloat16)

    one = {
        "fmain": fmain,
        "ftin": ftin,
        "eohin": eohin,
        "cst": cst,
        "cstb": cstb,
    }
    return [dict(one) for _ in range(NCORES)]


def kernel(features, labels):
    nc = _build()
    in_maps = _make_in_maps(features, labels)
    out = run_bass_kernel_spmd(nc, in_maps, core_ids=list(range(NCORES)))
    return np.float32(out.results[0]["res"][0, 0])
